# revision 4
# baseline (speedup 1.0000x reference)
"""Trainium2 Bass kernel for nn_BinTreeNetwork (binary-tree MLP expansion).

Strategy
--------
The reference is a 21-level binary-tree expansion ending at a (2,)^21 x 32
fp32 output (256 MB). Everything is linear; in flat row terms each level
doubles the rows via out'[r'] = out[r' mod M] + C[r'], C = res @ Wo_i.T,
so the final output row r is

  out[r] = o_L0[r mod 2^L0] + sum_{i=L0..20} C_i[r mod 2^(i+1)] + bias.

Row-index bits make a mod-8 row sharding communication-free: core q owns
rows ≡ q (mod 8). The host computes the tiny exact state path (L/R, 2
floats per row) through all levels and the o-accumulator only up to level
L0=17, then ships per core:

  - stack: the packed res planes of levels 17..20, each tiled along
    columns to the full output width (wrap r mod 2^(i+1) is a pure
    column-period in the packed layout) and stacked on the partition
    axis -> [32, 65536] fp16. One K=32 matmul against a block-diagonal
    weight stack computes ALL four levels' C contributions at once.
  - o17: the level-17 o accumulator (+out_bias folded) packed
    [128, 4096] fp16; a K=128 identity matmul accumulates it into the
    same PSUM bank, so the tree-broadcast add costs no vector cycles.

Packed layouts ("mod-4 stacked planes"): o/out tiles are [128, M/4]:
partition 32*(row%4)+plane, column row//4; res tiles are [8, M/2]:
partition 2*(row%4)+plane. All wraps become column slices, every engine
op runs at full width, and all DMAs are contiguous.

All DRAM traffic is fp16 (~21 MiB/core vs 50 fp32): stack 4 + o 1 +
out 16. PSUM accumulation stays fp32; fp16 rounding of inputs/outputs
costs ~5e-4 relative error (gate is 2e-2). PSUM->SBUF fp16 conversion
is split between the Act and DVE engines; out-DMAs alternate between
the SP and Act queues.
"""
import os
import numpy as np
from contextlib import ExitStack

import concourse.bass as bass
import concourse.bacc as bacc
import concourse.mybir as mybir
import concourse.tile as tile
from concourse.bass_utils import run_bass_kernel_spmd

T = 21
L0 = int(os.environ.get("BT_L0", "17"))
CHUNK = int(os.environ.get("BT_CHUNK", "2048"))
PIECE = int(os.environ.get("BT_PIECE", "8192"))  # stack-DMA staging piece
N = 1 << (T - 5)          # 65536 final packed cols per core
F16 = mybir.dt.float16
F32 = mybir.dt.float32

_CACHE = {}


# ---------------- host-side exact precompute ----------------

def _host_precompute(inputs):
    x = inputs["x"].astype(np.float32)
    L = (x @ inputs["in_left_layer"].T + inputs["in_left_bias"]).reshape(1, 2).astype(np.float32)
    R = (x @ inputs["in_right_layer"].T + inputs["in_right_bias"]).reshape(1, 2).astype(np.float32)
    out = (x @ inputs["out_layer0"].T).reshape(1, 32).astype(np.float32)
    res_levels = []
    o_L0 = None
    for i in range(T):
        M = L.shape[0]
        if i == L0:
            o_L0 = out
        if M == 1:
            res = np.array([[L[0, 0], R[0, 0]], [L[0, 1], R[0, 1]]], np.float32)
        else:
            res = np.concatenate([L[: M // 2], R[: M // 2], L[M // 2 :], R[M // 2 :]], axis=0)
        if i >= L0:
            res_levels.append(res)
        else:
            C = res @ inputs["out_layers"][i].T
            out = np.concatenate([out + C[:M], out + C[M:]], axis=0)
        if i < T - 1:  # last level's L/R states are unused
            L = res @ inputs["tree_left_layers"][i].T + inputs["tree_left_biases"][i]
            R = res @ inputs["tree_right_layers"][i].T + inputs["tree_right_biases"][i]
    o_L0 = o_L0 + inputs["out_bias"].astype(np.float32)[None, :]
    return o_L0, res_levels


def _pack_o_mod4(o_rows):
    M = o_rows.shape[0]
    return np.ascontiguousarray(
        o_rows.reshape(M // 4, 4, 32).transpose(1, 2, 0).reshape(128, M // 4))


def _unpack_o_mod4(t):
    Mc = t.shape[1]
    return np.ascontiguousarray(
        t.reshape(4, 32, Mc).transpose(2, 0, 1).reshape(4 * Mc, 32))


def _pack_res8(res):
    m2 = res.shape[0]
    cols = m2 // 4
    return np.ascontiguousarray(
        res.reshape(cols, 4, 2).transpose(1, 2, 0).reshape(8, cols))


def _make_lhsT(Wo):
    t = np.zeros((8, 128), np.float32)
    for b in range(4):
        for f in range(2):
            t[2 * b + f, 32 * b: 32 * (b + 1)] = Wo[:, f]
    return t


# ---------------- device program ----------------

def _build_nc():
    nlev = T - L0
    K = 8 * nlev
    ocols = 1 << (L0 - 5)

    nc = bacc.Bacc("TRN2", target_bir_lowering=False, debug=False,
                   enable_asserts=True, num_devices=8)

    wc_d = nc.dram_tensor("wc", [K, 128], F16, kind="ExternalInput").ap()
    id_d = nc.dram_tensor("ident", [128, 128], F16, kind="ExternalInput").ap()
    o_d = nc.dram_tensor("o_init", [128, ocols], F16, kind="ExternalInput").ap()
    stack_d = nc.dram_tensor("stack", [K, N], F16, kind="ExternalInput").ap()
    out_d = nc.dram_tensor("out", [128, N], F16, kind="ExternalOutput").ap()

    with tile.TileContext(nc, trace_sim=False) as tc:
        ctx = ExitStack()
        with ctx:
            const_pool = ctx.enter_context(tc.tile_pool(name="consts", bufs=1))
            stack_pool = ctx.enter_context(tc.tile_pool(name="stk", bufs=N // PIECE))
            outc_pool = ctx.enter_context(tc.tile_pool(name="outc", bufs=3))
            psum_pool = ctx.enter_context(tc.tile_pool(name="ps", bufs=2, space="PSUM"))

            wc_sb = const_pool.tile([K, 128], F16, name="wc_sb")
            nc.scalar.dma_start(out=wc_sb[:], in_=wc_d[:])
            id_sb = const_pool.tile([128, 128], F16, name="id_sb")
            nc.scalar.dma_start(out=id_sb[:], in_=id_d[:])

            # o state on the SP queue so it lands in parallel with piece 0
            o_sb = const_pool.tile([128, ocols], F16, name="o_sb")
            nc.sync.dma_start(out=o_sb[:], in_=o_d[:])

            # stack pieces stream via the (otherwise idle) Pool SWDGE queue,
            # emitted with one piece of lookahead relative to the chunk loop
            pieces = [None] * (N // PIECE)

            def need_piece(pi):
                if pi < len(pieces) and pieces[pi] is None:
                    pt = stack_pool.tile([K, PIECE], F16, name=f"stk{pi}", tag="stk")
                    nc.gpsimd.dma_start(
                        out=pt[:], in_=stack_d[:, pi * PIECE:(pi + 1) * PIECE])
                    pieces[pi] = pt

            need_piece(0)

            # Act/DVE split of the PSUM->fp16 staging copy, balanced by clock
            a_cols = (CHUNK * 5 // 9 + 63) & ~63
            nch = 0
            for c0 in range(0, N, CHUNK):
                c1 = c0 + CHUNK
                need_piece(c0 // PIECE + 1)
                pt = pieces[c0 // PIECE]
                pbase = c0 % PIECE
                obase = c0 % ocols
                ps = psum_pool.tile([128, CHUNK], F32, name=f"ps{c0}", tag="ps")
                for s in range(0, CHUNK, 512):
                    nc.tensor.matmul(ps[:, s:s + 512], wc_sb[:],
                                     pt[:, pbase + s:pbase + s + 512],
                                     start=True, stop=False)
                    nc.tensor.matmul(ps[:, s:s + 512], id_sb[:],
                                     o_sb[:, obase + s:obase + s + 512],
                                     start=False, stop=True)
                ot = outc_pool.tile([128, CHUNK], F16, name=f"ot{c0}", tag="outc")
                nc.scalar.copy(ot[:, 0:a_cols], ps[:, 0:a_cols])
                nc.vector.tensor_scalar_add(ot[:, a_cols:CHUNK], ps[:, a_cols:CHUNK], 0.0)
                eng = nc.sync if nch % 2 == 0 else nc.scalar
                eng.dma_start(out=out_d[:, c0:c1], in_=ot[:])
                nch += 1

    nc.compile()
    return nc


# ---------------- entry point ----------------

def kernel(**inputs):
    inputs = {k: np.asarray(v) for k, v in inputs.items()}
    o_L0, res_levels = _host_precompute(inputs)

    if "nc" not in _CACHE:
        _CACHE["nc"] = _build_nc()
    nc = _CACHE["nc"]

    nlev = T - L0
    wc = np.concatenate(
        [_make_lhsT(np.asarray(inputs["out_layers"][L0 + li], np.float32))
         for li in range(nlev)], axis=0).astype(np.float16)
    ident = np.eye(128, dtype=np.float16)

    in_maps = []
    for q in range(8):
        rows = []
        for li in range(nlev):
            t8 = _pack_res8(np.ascontiguousarray(res_levels[li][q::8]))
            rows.append(np.tile(t8, (1, N // t8.shape[1])))
        stackq = np.ascontiguousarray(np.concatenate(rows, axis=0), np.float16)
        m = {"wc": wc, "ident": ident, "stack": stackq,
             "o_init": _pack_o_mod4(o_L0[q::8]).astype(np.float16)}
        in_maps.append(m)

    res = run_bass_kernel_spmd(nc, in_maps, list(range(8)))

    full = np.empty((2 ** T, 32), np.float32)
    for q in range(8):
        full[q::8] = _unpack_o_mod4(res.results[q]["out"].astype(np.float32))
    return full.reshape((2,) * T + (32,))


# revision 5
# speedup vs baseline: 1.0013x; 1.0013x over previous
"""Trainium2 Bass kernel for nn_BinTreeNetwork (binary-tree MLP expansion).

Strategy
--------
The reference is a 21-level binary-tree expansion ending at a (2,)^21 x 32
fp32 output (256 MB). Everything is linear; in flat row terms each level
doubles the rows via out'[r'] = out[r' mod M] + C[r'], C = res @ Wo_i.T,
so the final output row r is

  out[r] = o_L0[r mod 2^L0] + sum_{i=L0..20} C_i[r mod 2^(i+1)] + bias.

Row-index bits make a mod-8 row sharding communication-free: core q owns
rows ≡ q (mod 8). The host computes the tiny exact state path (L/R, 2
floats per row) through all levels and the o-accumulator only up to level
L0=17, then ships per core:

  - stack: the packed res planes of levels 17..20, each tiled along
    columns to the full output width (wrap r mod 2^(i+1) is a pure
    column-period in the packed layout) and stacked on the partition
    axis -> [32, 65536] fp16. One K=32 matmul against a block-diagonal
    weight stack computes ALL four levels' C contributions at once.
  - o17: the level-17 o accumulator (+out_bias folded) packed
    [128, 4096] fp16; a K=128 identity matmul accumulates it into the
    same PSUM bank, so the tree-broadcast add costs no vector cycles.

Packed layouts ("mod-4 stacked planes"): o/out tiles are [128, M/4]:
partition 32*(row%4)+plane, column row//4; res tiles are [8, M/2]:
partition 2*(row%4)+plane. All wraps become column slices, every engine
op runs at full width, and all DMAs are contiguous.

All DRAM traffic is fp16 (~21 MiB/core vs 50 fp32): stack 4 + o 1 +
out 16. PSUM accumulation stays fp32; fp16 rounding of inputs/outputs
costs ~5e-4 relative error (gate is 2e-2). PSUM->SBUF fp16 conversion
is split between the Act and DVE engines; out-DMAs alternate between
the SP and Act queues.
"""
import os
import numpy as np
from contextlib import ExitStack

import concourse.bass as bass
import concourse.bacc as bacc
import concourse.mybir as mybir
import concourse.tile as tile
from concourse.bass_utils import run_bass_kernel_spmd

T = 21
L0 = int(os.environ.get("BT_L0", "17"))
CHUNK = int(os.environ.get("BT_CHUNK", "2048"))
PIECE = int(os.environ.get("BT_PIECE", "8192"))  # stack-DMA staging piece
N = 1 << (T - 5)          # 65536 final packed cols per core
if os.environ.get("BT_DT", "f16") == "bf16":
    import ml_dtypes
    F16 = mybir.dt.bfloat16
    NP16 = ml_dtypes.bfloat16
else:
    F16 = mybir.dt.float16
    NP16 = np.float16
F32 = mybir.dt.float32

_CACHE = {}


# ---------------- host-side exact precompute ----------------

def _host_precompute(inputs):
    x = inputs["x"].astype(np.float32)
    L = (x @ inputs["in_left_layer"].T + inputs["in_left_bias"]).reshape(1, 2).astype(np.float32)
    R = (x @ inputs["in_right_layer"].T + inputs["in_right_bias"]).reshape(1, 2).astype(np.float32)
    out = (x @ inputs["out_layer0"].T).reshape(1, 32).astype(np.float32)
    res_levels = []
    o_L0 = None
    for i in range(T):
        M = L.shape[0]
        if i == L0:
            o_L0 = out
        if M == 1:
            res = np.array([[L[0, 0], R[0, 0]], [L[0, 1], R[0, 1]]], np.float32)
        else:
            res = np.concatenate([L[: M // 2], R[: M // 2], L[M // 2 :], R[M // 2 :]], axis=0)
        if i >= L0:
            res_levels.append(res)
        else:
            C = res @ inputs["out_layers"][i].T
            out = np.concatenate([out + C[:M], out + C[M:]], axis=0)
        if i < T - 1:  # last level's L/R states are unused
            L = res @ inputs["tree_left_layers"][i].T + inputs["tree_left_biases"][i]
            R = res @ inputs["tree_right_layers"][i].T + inputs["tree_right_biases"][i]
    o_L0 = o_L0 + inputs["out_bias"].astype(np.float32)[None, :]
    return o_L0, res_levels


def _pack_o_mod4(o_rows):
    M = o_rows.shape[0]
    return np.ascontiguousarray(
        o_rows.reshape(M // 4, 4, 32).transpose(1, 2, 0).reshape(128, M // 4))


def _unpack_o_mod4(t):
    Mc = t.shape[1]
    return np.ascontiguousarray(
        t.reshape(4, 32, Mc).transpose(2, 0, 1).reshape(4 * Mc, 32))


def _pack_res8(res):
    m2 = res.shape[0]
    cols = m2 // 4
    return np.ascontiguousarray(
        res.reshape(cols, 4, 2).transpose(1, 2, 0).reshape(8, cols))


def _make_lhsT(Wo):
    t = np.zeros((8, 128), np.float32)
    for b in range(4):
        for f in range(2):
            t[2 * b + f, 32 * b: 32 * (b + 1)] = Wo[:, f]
    return t


# ---------------- device program ----------------

def _build_nc():
    nlev = T - L0
    K = 8 * nlev
    ocols = 1 << (L0 - 5)

    nc = bacc.Bacc("TRN2", target_bir_lowering=False, debug=False,
                   enable_asserts=True, num_devices=8)

    wc_d = nc.dram_tensor("wc", [K, 128], F16, kind="ExternalInput").ap()
    id_d = nc.dram_tensor("ident", [128, 128], F16, kind="ExternalInput").ap()
    o_d = nc.dram_tensor("o_init", [128, ocols], F16, kind="ExternalInput").ap()
    stack_d = nc.dram_tensor("stack", [K, N], F16, kind="ExternalInput").ap()
    out_d = nc.dram_tensor("out", [128, N], F16, kind="ExternalOutput").ap()

    with tile.TileContext(nc, trace_sim=False) as tc:
        ctx = ExitStack()
        with ctx:
            const_pool = ctx.enter_context(tc.tile_pool(name="consts", bufs=1))
            stack_pool = ctx.enter_context(tc.tile_pool(name="stk", bufs=N // PIECE))
            outc_pool = ctx.enter_context(tc.tile_pool(name="outc", bufs=3))
            psum_pool = ctx.enter_context(tc.tile_pool(name="ps", bufs=2, space="PSUM"))

            wc_sb = const_pool.tile([K, 128], F16, name="wc_sb")
            nc.scalar.dma_start(out=wc_sb[:], in_=wc_d[:])
            id_sb = const_pool.tile([128, 128], F16, name="id_sb")
            nc.scalar.dma_start(out=id_sb[:], in_=id_d[:])

            # o state on the SP queue so it lands in parallel with piece 0
            o_sb = const_pool.tile([128, ocols], F16, name="o_sb")
            nc.sync.dma_start(out=o_sb[:], in_=o_d[:])

            # stack pieces stream via the (otherwise idle) Pool SWDGE queue,
            # emitted with one piece of lookahead relative to the chunk loop
            pieces = [None] * (N // PIECE)

            def need_piece(pi):
                if pi < len(pieces) and pieces[pi] is None:
                    pt = stack_pool.tile([K, PIECE], F16, name=f"stk{pi}", tag="stk")
                    nc.gpsimd.dma_start(
                        out=pt[:], in_=stack_d[:, pi * PIECE:(pi + 1) * PIECE])
                    pieces[pi] = pt

            need_piece(0)

            # Act/DVE split of the PSUM->fp16 staging copy, balanced by clock
            a_cols = (CHUNK * 5 // 9 + 63) & ~63
            nch = 0
            for c0 in range(0, N, CHUNK):
                c1 = c0 + CHUNK
                need_piece(c0 // PIECE + 1)
                pt = pieces[c0 // PIECE]
                pbase = c0 % PIECE
                obase = c0 % ocols
                ps = psum_pool.tile([128, CHUNK], F32, name=f"ps{c0}", tag="ps")
                for s in range(0, CHUNK, 512):
                    nc.tensor.matmul(ps[:, s:s + 512], wc_sb[:],
                                     pt[:, pbase + s:pbase + s + 512],
                                     start=True, stop=False)
                    nc.tensor.matmul(ps[:, s:s + 512], id_sb[:],
                                     o_sb[:, obase + s:obase + s + 512],
                                     start=False, stop=True)
                ot = outc_pool.tile([128, CHUNK], F16, name=f"ot{c0}", tag="outc")
                nc.scalar.copy(ot[:, 0:a_cols], ps[:, 0:a_cols])
                nc.vector.tensor_scalar_add(ot[:, a_cols:CHUNK], ps[:, a_cols:CHUNK], 0.0)
                eng = nc.sync if nch % 2 == 0 else nc.scalar
                eng.dma_start(out=out_d[:, c0:c1], in_=ot[:])
                nch += 1

    nc.compile()
    return nc


# ---------------- entry point ----------------

def kernel(**inputs):
    inputs = {k: np.asarray(v) for k, v in inputs.items()}
    o_L0, res_levels = _host_precompute(inputs)

    if "nc" not in _CACHE:
        _CACHE["nc"] = _build_nc()
    nc = _CACHE["nc"]

    nlev = T - L0
    wc = np.concatenate(
        [_make_lhsT(np.asarray(inputs["out_layers"][L0 + li], np.float32))
         for li in range(nlev)], axis=0).astype(NP16)
    ident = np.eye(128, dtype=NP16)

    in_maps = []
    for q in range(8):
        rows = []
        for li in range(nlev):
            t8 = _pack_res8(np.ascontiguousarray(res_levels[li][q::8]))
            rows.append(np.tile(t8, (1, N // t8.shape[1])))
        stackq = np.ascontiguousarray(np.concatenate(rows, axis=0)).astype(NP16)
        m = {"wc": wc, "ident": ident, "stack": stackq,
             "o_init": _pack_o_mod4(o_L0[q::8]).astype(NP16)}
        in_maps.append(m)

    res = run_bass_kernel_spmd(nc, in_maps, list(range(8)))

    full = np.empty((2 ** T, 32), np.float32)
    for q in range(8):
        full[q::8] = _unpack_o_mod4(res.results[q]["out"].astype(np.float32))
    return full.reshape((2,) * T + (32,))


# revision 10
# speedup vs baseline: 1.5801x; 1.5781x over previous
"""Trainium2 Bass kernel for nn_BinTreeNetwork (binary-tree MLP expansion).

Strategy
--------
The reference is a 21-level binary-tree expansion ending at a (2,)^21 x 32
fp32 output (256 MB). Everything is linear; in flat row terms each level
doubles the rows via out'[r'] = out[r' mod M] + C[r'], C = res @ Wo_i.T,
so the final output row r is

  out[r] = o_L0[r mod 2^L0] + sum_{i=L0..20} C_i[r mod 2^(i+1)] + bias.

Row-index bits make a mod-8 row sharding communication-free: core q owns
rows ≡ q (mod 8). The host computes the tiny exact state path (L/R, 2
floats per row) through all levels and the o-accumulator only up to level
L0=17, then ships per core:

  - stack: the packed res planes of levels 17..20, each tiled along
    columns to the full output width (wrap r mod 2^(i+1) is a pure
    column-period in the packed layout) and stacked on the partition
    axis -> [32, 65536] fp16. One K=32 matmul against a block-diagonal
    weight stack computes ALL four levels' C contributions at once.
  - o17: the level-17 o accumulator (+out_bias folded) packed
    [128, 4096] fp16; a K=128 identity matmul accumulates it into the
    same PSUM bank, so the tree-broadcast add costs no vector cycles.

Packed layouts ("mod-4 stacked planes"): o/out tiles are [128, M/4]:
partition 32*(row%4)+plane, column row//4; res tiles are [8, M/2]:
partition 2*(row%4)+plane. All wraps become column slices, every engine
op runs at full width, and all DMAs are contiguous.

All DRAM traffic is fp16 (~21 MiB/core vs 50 fp32): stack 4 + o 1 +
out 16. PSUM accumulation stays fp32; fp16 rounding of inputs/outputs
costs ~5e-4 relative error (gate is 2e-2). PSUM->SBUF fp16 conversion
is split between the Act and DVE engines; out-DMAs alternate between
the SP and Act queues.
"""
import os
import numpy as np
from contextlib import ExitStack

import concourse.bass as bass
import concourse.bacc as bacc
import concourse.mybir as mybir
import concourse.tile as tile
from concourse.bass_utils import run_bass_kernel_spmd

T = 21
L0 = int(os.environ.get("BT_L0", "17"))
CHUNK = int(os.environ.get("BT_CHUNK", "2048"))
PIECE = int(os.environ.get("BT_PIECE", "8192"))  # stack-DMA staging piece
N = 1 << (T - 5)          # 65536 final packed cols per core
if os.environ.get("BT_DT", "f16") == "bf16":
    import ml_dtypes
    F16 = mybir.dt.bfloat16
    NP16 = ml_dtypes.bfloat16
else:
    F16 = mybir.dt.float16
    NP16 = np.float16
F32 = mybir.dt.float32

_CACHE = {}


# ---------------- host-side exact precompute ----------------

def _host_precompute(inputs):
    x = inputs["x"].astype(np.float32)
    L = (x @ inputs["in_left_layer"].T + inputs["in_left_bias"]).reshape(1, 2).astype(np.float32)
    R = (x @ inputs["in_right_layer"].T + inputs["in_right_bias"]).reshape(1, 2).astype(np.float32)
    out = (x @ inputs["out_layer0"].T).reshape(1, 32).astype(np.float32)
    res_levels = []
    o_L0 = None
    for i in range(T):
        M = L.shape[0]
        if i == L0:
            o_L0 = out
        if M == 1:
            res = np.array([[L[0, 0], R[0, 0]], [L[0, 1], R[0, 1]]], np.float32)
        else:
            res = np.concatenate([L[: M // 2], R[: M // 2], L[M // 2 :], R[M // 2 :]], axis=0)
        if i >= L0:
            res_levels.append(res)
        else:
            C = res @ inputs["out_layers"][i].T
            out = np.concatenate([out + C[:M], out + C[M:]], axis=0)
        if i < T - 1:  # last level's L/R states are unused
            L = res @ inputs["tree_left_layers"][i].T + inputs["tree_left_biases"][i]
            R = res @ inputs["tree_right_layers"][i].T + inputs["tree_right_biases"][i]
    o_L0 = o_L0 + inputs["out_bias"].astype(np.float32)[None, :]
    return o_L0, res_levels


def _pack_o_mod4(o_rows):
    M = o_rows.shape[0]
    return np.ascontiguousarray(
        o_rows.reshape(M // 4, 4, 32).transpose(1, 2, 0).reshape(128, M // 4))


def _unpack_o_mod4(t):
    Mc = t.shape[1]
    return np.ascontiguousarray(
        t.reshape(4, 32, Mc).transpose(2, 0, 1).reshape(4 * Mc, 32))


def _pack_res8(res):
    m2 = res.shape[0]
    cols = m2 // 4
    return np.ascontiguousarray(
        res.reshape(cols, 4, 2).transpose(1, 2, 0).reshape(8, cols))


def _make_lhsT(Wo):
    t = np.zeros((8, 128), np.float32)
    for b in range(4):
        for f in range(2):
            t[2 * b + f, 32 * b: 32 * (b + 1)] = Wo[:, f]
    return t


# ---------------- device program ----------------

def _build_nc():
    nlev = T - L0
    K = 8 * nlev
    assert K == 32, "row-group interleave assumes K=32 (L0=17)"
    ocols = 1 << (L0 - 5)
    NT = N // 4          # "tall" column count (4 row-group stripes)
    PT = PIECE // 4

    nc = bacc.Bacc("TRN2", target_bir_lowering=False, debug=False,
                   enable_asserts=True, num_devices=8)

    wc_d = nc.dram_tensor("wc", [128, 128], F16, kind="ExternalInput").ap()
    o_d = nc.dram_tensor("o_init", [128, ocols], F16, kind="ExternalInput").ap()
    stack_d = nc.dram_tensor("stack", [128, NT], F16, kind="ExternalInput").ap()
    out_d = nc.dram_tensor("out", [128, N], F16, kind="ExternalOutput").ap()

    with tile.TileContext(nc, trace_sim=False) as tc:
        ctx = ExitStack()
        with ctx:
            const_pool = ctx.enter_context(tc.tile_pool(name="consts", bufs=1))
            stack_pool = ctx.enter_context(tc.tile_pool(name="stk", bufs=N // PIECE))
            outc_pool = ctx.enter_context(tc.tile_pool(name="outc", bufs=3))
            tmp_pool = ctx.enter_context(tc.tile_pool(name="tmp", bufs=3))
            psum_pool = ctx.enter_context(tc.tile_pool(name="ps", bufs=2, space="PSUM"))

            wc_sb = const_pool.tile([128, 128], F16, name="wc_sb")
            nc.scalar.dma_start(out=wc_sb[:], in_=wc_d[:])

            # o state on the SP queue so it lands in parallel with piece 0
            o_sb = const_pool.tile([128, ocols], F16, name="o_sb")
            nc.sync.dma_start(out=o_sb[:], in_=o_d[:])

            # stack pieces stream via the Act HWDGE queue, emitted with one
            # piece of lookahead relative to the chunk loop
            pieces = [None] * (N // PIECE)

            def need_piece(pi):
                if pi < len(pieces) and pieces[pi] is None:
                    pt = stack_pool.tile([128, PT], F16, name=f"stk{pi}", tag="stk")
                    nc.scalar.dma_start(
                        out=pt[:], in_=stack_d[:, pi * PT:(pi + 1) * PT])
                    pieces[pi] = pt

            need_piece(0)

            # 3-way split of the psum+o -> fp16 elementwise stage:
            #   [0:xv]        DVE fused scalar_tensor_tensor from PSUM
            #   [xv:xv+za]    Act copy psum->fp16 tmp, DVE tensor_tensor add
            #   [xv+za:CHUNK] Act copy psum->fp16 tmp, Pool tensor_tensor add
            xv = int(os.environ.get("BT_XV", "768"))
            za = int(os.environ.get("BT_ZA", "640"))
            add = mybir.AluOpType.add
            nch = 0
            for c0 in range(0, N, CHUNK):
                c1 = c0 + CHUNK
                need_piece(c0 // PIECE + 1)
                pt = pieces[c0 // PIECE]
                tc0 = c0 // 4
                pbase = tc0 % PT
                obase = c0 % ocols
                ps = psum_pool.tile([128, CHUNK], F32, name=f"ps{c0}", tag="ps")
                for g in range(4):
                    nc.tensor.matmul(ps[:, 512 * g:512 * (g + 1)],
                                     wc_sb[32 * g:32 * (g + 1), :],
                                     pt[32 * g:32 * (g + 1), pbase:pbase + 512],
                                     start=True, stop=True,
                                     tile_position=(32 * g, 0))
                ot = outc_pool.tile([128, CHUNK], F16, name=f"ot{c0}", tag="outc")
                tmp = tmp_pool.tile([128, CHUNK - xv], F16, name=f"tm{c0}", tag="tmp")
                nc.vector.scalar_tensor_tensor(
                    ot[:, 0:xv], ps[:, 0:xv], 0.0,
                    o_sb[:, obase:obase + xv], add, add)
                nc.scalar.copy(tmp[:, 0:CHUNK - xv], ps[:, xv:CHUNK])
                nc.vector.tensor_tensor(
                    ot[:, xv:xv + za], tmp[:, 0:za],
                    o_sb[:, obase + xv:obase + xv + za], add)
                nc.gpsimd.tensor_tensor(
                    ot[:, xv + za:CHUNK], tmp[:, za:CHUNK - xv],
                    o_sb[:, obase + xv + za:obase + CHUNK], add)
                nc.sync.dma_start(out=out_d[:, c0:c1], in_=ot[:])
                nch += 1

    nc.compile()
    return nc


# ---------------- entry point ----------------

def kernel(**inputs):
    inputs = {k: np.asarray(v) for k, v in inputs.items()}
    o_L0, res_levels = _host_precompute(inputs)

    if "nc" not in _CACHE:
        _CACHE["nc"] = _build_nc()
    nc = _CACHE["nc"]

    nlev = T - L0
    K = 8 * nlev
    wc = np.concatenate(
        [_make_lhsT(np.asarray(inputs["out_layers"][L0 + li], np.float32))
         for li in range(nlev)], axis=0)
    wc_tall = np.tile(wc, (128 // K, 1)).astype(NP16)

    in_maps = []
    for q in range(8):
        rows = []
        for li in range(nlev):
            t8 = _pack_res8(np.ascontiguousarray(res_levels[li][q::8]))
            rows.append(np.tile(t8, (1, N // t8.shape[1])))
        stackq = np.concatenate(rows, axis=0)
        # interleave 512-col stripes into 4 row-group partition blocks:
        # tall[32g+k, 512t+s] = stack[k, 2048t+512g+s]
        tall = np.ascontiguousarray(
            stackq.reshape(K, N // 2048, 4, 512).transpose(2, 0, 1, 3)
            .reshape(4 * K, N // 4)).astype(NP16)
        m = {"wc": wc_tall, "stack": tall,
             "o_init": _pack_o_mod4(o_L0[q::8]).astype(NP16)}
        in_maps.append(m)

    res = run_bass_kernel_spmd(nc, in_maps, list(range(8)))

    full = np.empty((2 ** T, 32), np.float32)
    for q in range(8):
        full[q::8] = _unpack_o_mod4(res.results[q]["out"].astype(np.float32))
    return full.reshape((2,) * T + (32,))
